# revision 7
# baseline (speedup 1.0000x reference)
"""CrossTransFormer attention kernel for 8x Trainium2 NeuronCores (Bass/Tile).

Problem (per batch b, B=8, C=773, P=4096):
    K = Wk @ Xk + bk            [C, P]
    V = Wv @ Xq + bv            [C, P]
    S[i, j] = sum_c K[c, i] * V[c, j] / sqrt(C)       (i, j over P)
    H = softmax(S, axis=i)
    out[k, j] = sum_i Xk[k, i] * H[i, j]              [C, P]

Sharding: data-parallel over batch, one batch per NeuronCore, no collectives.

Algebraic restructure (saves one full projection + all weight transposes):
    S = Xk^T (Wk^T Wv) Xq + u 1^T + 1 w^T   with u = Xk^T (Wk^T bv);
    the j-indexed w term is constant along the softmax axis i and cancels
    exactly -> dropped.
  GT = Wv^T Wk is computed on the PE with both weights in NATURAL layout,
  w1 = Wk^T bv rides along; both fold into the A-projection
  A = G Xq + w1 1^T.  The A-proj lhsT tiles are zero-padded to c1=896 so
  every projection chain emits full-128-partition PSUM tiles with clean
  zeros in the ragged rows.

Xk is loaded ct-major in half-rows (8 KB DRAM lines, ~2x the per-line DMA
throughput of 2 KB j-block lines), cast on gpsimd, with the c2 tail
written at partition bases 0/32/64 (three replicas) plus an all-ones row
at partition 96.

QT = Xk^T is produced by the DMA XBAR transpose engine (14 batched
dma_start_transpose calls), entirely off the PE, each fired as soon as its
(c-tile, half) is cast.  The tail-block transpose carries the ones row to
qt col 864 for free, so the ragged out-chain lands softmax sums on PSUM
partition 96 (legal compute-engine base).  No plain SBUF->SBUF HWDGE DMAs
are issued anywhere (XBAR-transpose || SBUF->SBUF DMA is a known HW
deadlock): the w1 row is computed directly into PSUM partition 5 by giving
the bias column M=6 (cols 0..4 zero), and the xq6 ones row comes from a
memset-1.0 + partial overwrite.

Fused phase D (per j-block of 512), everything SBUF-resident:
  A-proj: 7 chains of 7 MMs -> ast[128, 7, 512] fp16; the c2-tail rows of
  ast are then replicated to partition bases 32/64.
  S-phase: 32 i-tiles in groups of 3: per chain 6 full K=128 MMs, then the
  three K=5 c2-tail MMs issue back-to-back into row-groups 0/32/64 so they
  pipeline in the PE array (~1 MM span instead of 3); ACT exp
  (scale=1/sqrt(C)) into es[128, 32, 512] fp16.
  out-phase: 7 k-tile chains of 32 accumulating MMs; the ragged chain
  (5 data rows + softmax-sum row from the qt ones-column at partition 96)
  runs FIRST so the reciprocal + partition-broadcast overlap the remaining
  chains; each chain is normalized (DVE) and DMA'd out as it finishes.
"""

import sys

sys.path.insert(0, "/opt/trn_rl_repo")

import numpy as np

import concourse.bacc as bacc
import concourse.mybir as mybir
import concourse.tile as tile
from concourse.bass_utils import run_bass_kernel_spmd

F32 = mybir.dt.float32
F16 = mybir.dt.float16

C = 773
PT = 128
CT = 7  # ceil(773 / 128) chunks of the channel dim
LC = C - (CT - 1) * PT  # 5 rows in the last chunk
JB = 512  # j-block width (one PSUM bank of fp32)
CW = CT * PT  # c1 padded to 896 for the zero-padded A-proj lhsT
TP = 112  # partitions covered by the tail-block transpose (16-aligned)
QW = 6 * PT + TP  # qt width: 6 full c-tiles + 112-col XBAR tail block
ONE_ROW = 96  # all-ones row in xk16 tile 6 -> qt col 768+96, sums partition


def build(P=4096, n_cores=8):
    NJ = P // JB
    IT = P // PT
    HP = P // 2
    SCALE = float(1.0 / np.sqrt(C))

    nc = bacc.Bacc("TRN2", target_bir_lowering=False, debug=False,
                   num_devices=n_cores)
    Xq = nc.dram_tensor("Xq", [C, P], F32, kind="ExternalInput")
    Xk = nc.dram_tensor("Xk", [C, P], F32, kind="ExternalInput")
    Wk = nc.dram_tensor("Wk", [C, C], F32, kind="ExternalInput")
    bk = nc.dram_tensor("bk", [C], F32, kind="ExternalInput")
    Wv = nc.dram_tensor("Wv", [C, C], F32, kind="ExternalInput")
    bv = nc.dram_tensor("bv", [C], F32, kind="ExternalInput")
    out = nc.dram_tensor("out", [C, P], F32, kind="ExternalOutput")
    del bk  # only enters via a softmax-invariant per-j term

    with tile.TileContext(nc) as tc:
        with tc.tile_pool(name="persist", bufs=1) as persist:
            # Xk fp16 resident, natural [c, p] layout: lhsT tiles for S.
            # Tile 6: rows 0..4 / 32..36 / 64..68 = three replicas of the
            # ragged c2 tail (for the row-group-packed S tail MMs), row 96
            # = all-ones (becomes the qt ones-column via the XBAR
            # transpose), rest zeros.
            xk16 = persist.tile([PT, CT, P], F16)
            # exp(S) for one j-block, [i-in-tile, it, j]
            es = persist.tile([PT, IT, JB], F16)
            # GT = Wv^T Wk [c2-part, ct2, c1] fp16, c1 zero-padded to 896
            g16 = persist.tile([PT, CT, CW], F16)
            # packed ragged lhsT: rows 0..4 = GT c2-ragged rows, row 5 = w1
            g6 = persist.tile([8, CW], F16)

            # PE warmup: dummy matmuls so the HAM clock-gate opens
            # (4/8 -> 8/8) while the first DMAs are in flight, and the
            # exp activation table loads before the main loop.
            wsb = tc.alloc_tile_pool(name="wsb", bufs=1)
            warm = wsb.tile([PT, JB], F16)
            nc.vector.memset(warm[:, :], 0.0)
            with tc.tile_pool(name="pswarm", bufs=4, space="PSUM") as pswarm:
                for i in range(30):
                    wps = pswarm.tile([PT, JB], F32, tag="wps",
                                      name=f"wps{i}")
                    nc.tensor.matmul(wps[:, :], warm[:, :PT], warm[:, :],
                                     start=True, stop=True,
                                     skip_group_check=True)
                wexp = wsb.tile([1, 16], F32)
                nc.scalar.activation(wexp[:], wps[:1, :16],
                                     mybir.ActivationFunctionType.Exp,
                                     scale=1.0)
            wsb.release()

            # zero-pad fills.  DVE: xk16 tile-6 zeros + ones row (early
            # consumers).  gpsimd: g16 + g6 (consumed later; boot overlaps).
            nc.vector.memset(xk16[:, CT - 1, :], 0.0)
            nc.vector.memset(xk16[ONE_ROW:ONE_ROW + 1, CT - 1, :], 1.0)
            nc.gpsimd.memset(g16[:, :, :], 0.0)
            nc.gpsimd.memset(g6[:, :], 0.0)

            # ---- Phase G: GT = Wv^T Wk and w1 = Wk^T bv on the PE ----
            with (
                tc.tile_pool(name="wload", bufs=1) as wload,
                tc.tile_pool(name="psg", bufs=4, space="PSUM") as psg,
            ):
                wk16 = wload.tile([PT, CT, C], F16, tag="wk16")
                wv16 = wload.tile([PT, CT, C], F16, tag="wv16")
                # bias columns, M=6 per o-tile: cols 0..4 zero, col 5 = bv
                # chunk -> the w1 chain emits w1 directly on PSUM row 5.
                bvcol = wload.tile([PT, CT, 6], F16, tag="bvcol")
                nc.vector.memset(bvcol[:, :, :], 0.0)
                # batched W loads: 3 dma_starts per weight into an f32
                # staging ring, DVE-cast to fp16
                for Wsrc, dst in ((Wk, wk16), (Wv, wv16)):
                    for lo in (0, 3):
                        ws = wload.tile([PT, 3, C], F32, tag="wstage")
                        nc.sync.dma_start(
                            ws[:, :, :],
                            Wsrc[lo * PT:(lo + 3) * PT, :].rearrange(
                                "(ct p) c -> p ct c", p=PT),
                        )
                        nc.vector.tensor_copy(dst[:, lo:lo + 3, :],
                                              ws[:, :, :])
                    wt = wload.tile([8, C], F32, tag="wtail")
                    nc.sync.dma_start(wt[:LC, :], Wsrc[(CT - 1) * PT:C, :])
                    nc.vector.tensor_copy(dst[:LC, CT - 1, :], wt[:LC, :])
                # bv chunks into bvcol[:, ot, 5] on the gpsimd software
                # queue (DRAM->SBUF, cast f32->f16)
                for ot in range(CT - 1):
                    nc.gpsimd.dma_start(
                        bvcol[:, ot, 5:6], bv[ot * PT:(ot + 1) * PT, None])
                nc.gpsimd.dma_start(bvcol[:LC, CT - 1, 5:6],
                                    bv[(CT - 1) * PT:C, None])
                # GT tiles: [c2-tile, c1-chunk], contract over o (7 tiles)
                for ct2 in range(CT):
                    pc2 = PT if ct2 < CT - 1 else LC
                    for h, (j0, j1) in enumerate(((0, JB), (JB, C))):
                        ps = psg.tile([PT, JB], F32, tag="psg")
                        for ot in range(CT):
                            po = PT if ot < CT - 1 else LC
                            nc.tensor.matmul(
                                ps[:pc2, :j1 - j0],
                                wv16[:po, ot, ct2 * PT:ct2 * PT + pc2],
                                wk16[:po, ot, j0:j1],
                                start=(ot == 0),
                                stop=(ot == CT - 1),
                            )
                        nc.vector.tensor_copy(g16[:pc2, ct2, j0:j1],
                                              ps[:pc2, :j1 - j0])
                # w1 row: lhsT = bvcol (M=6, cols 0..4 zero) -> psum rows
                # 0..4 zero, row 5 = w1.  Copy rows 0..5 into g6 FIRST,
                # then overwrite rows 0..4 with the GT ragged rows (WAW
                # dep keeps the order).
                for h, (j0, j1) in enumerate(((0, JB), (JB, C))):
                    ps = psg.tile([8, JB], F32, tag="psw")
                    for ot in range(CT):
                        po = PT if ot < CT - 1 else LC
                        nc.tensor.matmul(
                            ps[:6, :j1 - j0],
                            bvcol[:po, ot, :],
                            wk16[:po, ot, j0:j1],
                            start=(ot == 0),
                            stop=(ot == CT - 1),
                        )
                    nc.vector.tensor_copy(g6[:6, j0:j1], ps[:6, :j1 - j0])
                nc.vector.tensor_copy(g6[:LC, :C], g16[:LC, CT - 1, :C])

            # QT pool reuses the space wload released.  qt[i, it, c]:
            # cols 0..767 from c-tiles 0..5, cols 768..879 from the
            # 112-row tail block (data rows 0..4 -> cols 768..772, ones
            # row 96 -> col 864, replicas at 800.. / 832.., zeros
            # elsewhere -- only cols 768..772 and 864 are consumed).
            qtp = tc.alloc_tile_pool(name="qtp", bufs=1)
            qt = qtp.tile([PT, IT, QW], F16)

            # pools that span phases B and D
            xqp = tc.alloc_tile_pool(name="xqp", bufs=2)
            xfp = tc.alloc_tile_pool(name="xfp", bufs=2)
            xtp = tc.alloc_tile_pool(name="xtp", bufs=2)
            xks = tc.alloc_tile_pool(name="xks", bufs=2)

            def load_xq(jb):
                js = slice(jb * JB, (jb + 1) * JB)
                xq16 = xqp.tile([PT, CT, JB], F16, tag="xq16",
                                name=f"xq16_{jb}")
                for lo in (0, 3):
                    xf = xfp.tile([PT, 3, JB], F32, tag="xstage",
                                  name=f"xqf{jb}_{lo}")
                    nc.sync.dma_start(
                        xf[:, :, :],
                        Xq[lo * PT:(lo + 3) * PT, js].rearrange(
                            "(ct p) c -> p ct c", p=PT),
                    )
                    nc.vector.tensor_copy(xq16[:, lo:lo + 3, :],
                                          xf[:, :, :])
                xt = xtp.tile([8, JB], F32, tag="xtail", name=f"xqt{jb}")
                nc.sync.dma_start(xt[:LC, :], Xq[(CT - 1) * PT:C, js])
                nc.vector.tensor_copy(xq16[:LC, CT - 1, :], xt[:LC, :])
                # packed ragged rhs: memset 1.0 (row 5 stays ones, rows
                # 6..7 hit zero g6 rows), rows 0..4 overwritten with the
                # Xq c2-ragged rows.
                xq6 = xqp.tile([8, JB], F16, tag="xq6", name=f"xq6_{jb}")
                nc.vector.memset(xq6[:, :], 1.0)
                nc.vector.tensor_copy(xq6[:LC, :], xq16[:LC, CT - 1, :])
                return xq16, xq6

            # ---- Phase B: stream Xk ct-major in half-rows (8 KB DRAM
            # lines), cast on gpsimd; fire each XBAR transpose as soon as
            # its (ct, half) is fully cast. ----
            xq_next = load_xq(0)
            for half in (0, 1):
                hs = slice(half * HP, (half + 1) * HP)
                ht = slice(half * (IT // 2), (half + 1) * (IT // 2))
                for ct in range(CT):
                    if ct < CT - 1:
                        xkf = xks.tile([PT, HP], F32, tag="xkstage",
                                       name=f"xkf{half}_{ct}")
                        nc.sync.dma_start(
                            xkf[:, :], Xk[ct * PT:(ct + 1) * PT, hs])
                        nc.gpsimd.tensor_copy(xk16[:, ct, hs], xkf[:, :])
                    else:
                        xkf = xks.tile([PT, HP], F32, tag="xkstage",
                                       name=f"xkf{half}_t")
                        nc.sync.dma_start(xkf[:LC, :], Xk[6 * PT:C, hs])
                        # three replicas of the c2 tail at bases 0/32/64
                        # (row-group-packed S tail MMs read them)
                        for base in (0, 32, 64):
                            nc.gpsimd.tensor_copy(
                                xk16[base:base + LC, ct, hs], xkf[:LC, :])
                for ct in range(CT):
                    pc = PT if ct < CT - 1 else TP
                    nc.sync.dma_start_transpose(
                        qt[:, ht, ct * PT:ct * PT + pc],
                        xk16[:pc, ct, hs],
                    )
            # staging no longer needed; free its SBUF for the phase-D pools
            xks.release()

            # ---- Phase D: fused A-projection + attention main loop ----
            with (
                tc.tile_pool(name="astp", bufs=1) as astp,
                tc.tile_pool(name="op", bufs=2) as op,
                tc.tile_pool(name="rp", bufs=1) as rp,
                tc.tile_pool(name="psA", bufs=2, space="PSUM") as psA,
                tc.tile_pool(name="psS", bufs=3, space="PSUM") as psS,
                tc.tile_pool(name="psO", bufs=3, space="PSUM") as psO,
            ):
                for jb in range(NJ):
                    js = slice(jb * JB, (jb + 1) * JB)
                    xq16, xq6 = xq_next
                    if jb < NJ - 1:
                        xq_next = load_xq(jb + 1)

                    # A-proj: A[:, jblock] = G @ Xq + w1 (ragged K=6 MM
                    # carries both the c2 tail and the bias row); the
                    # zero-padded lhsT makes all 128 psum rows valid
                    ast = astp.tile([PT, CT, JB], F16, tag="ast",
                                    name=f"ast{jb}")
                    for ot in range(CT):
                        ps = psA.tile([PT, JB], F32, tag="a",
                                      name=f"a{jb}_{ot}")
                        for ct2 in range(CT - 1):
                            nc.tensor.matmul(
                                ps[:, :],
                                g16[:, ct2, ot * PT:(ot + 1) * PT],
                                xq16[:, ct2, :],
                                start=(ct2 == 0),
                                stop=False,
                                skip_group_check=True,
                            )
                        nc.tensor.matmul(
                            ps[:, :],
                            g6[:LC + 1, ot * PT:(ot + 1) * PT],
                            xq6[:LC + 1, :],
                            start=False,
                            stop=True,
                            skip_group_check=True,
                        )
                        nc.vector.tensor_copy(ast[:, ot, :], ps[:, :])
                    # replicate the ast c2 tail to bases 32/64 for the
                    # row-group-packed S tail MMs
                    for base in (32, 64):
                        nc.vector.tensor_copy(
                            ast[base:base + LC, CT - 1, :],
                            ast[:LC, CT - 1, :])

                    # S-phase: groups of 3 chains; 6 full K=128 MMs per
                    # chain, then the three K=5 tail MMs pipeline in
                    # row-groups 0/32/64
                    for t0 in range(0, IT, 3):
                        grp = range(t0, min(t0 + 3, IT))
                        pss = {}
                        for t in grp:
                            ts = slice(t * PT, (t + 1) * PT)
                            ps_s = psS.tile([PT, JB], F32, tag="s",
                                            name=f"s{jb}_{t}")
                            pss[t] = ps_s
                            for ct in range(CT - 1):
                                nc.tensor.matmul(
                                    ps_s[:, :],
                                    xk16[:, ct, ts],
                                    ast[:, ct, :],
                                    start=(ct == 0),
                                    stop=False,
                                    skip_group_check=True,
                                )
                        for gi, t in enumerate(grp):
                            ts = slice(t * PT, (t + 1) * PT)
                            base = 32 * gi
                            nc.tensor.matmul(
                                pss[t][:, :],
                                xk16[base:base + LC, CT - 1, ts],
                                ast[base:base + LC, CT - 1, :],
                                start=False,
                                stop=True,
                                skip_group_check=True,
                            )
                        for t in grp:
                            nc.scalar.activation(
                                es[:, t, :], pss[t][:],
                                mybir.ActivationFunctionType.Exp,
                                scale=SCALE,
                            )

                    # out-phase: ragged chain (data rows 768..772 + sums
                    # row from the qt ones column at partition 96) FIRST,
                    # then full chains
                    bc = None
                    for kt in (CT - 1,) + tuple(range(CT - 1)):
                        mk = ONE_ROW + 1 if kt == CT - 1 else PT
                        ps_o = psO.tile([PT, JB], F32, tag="o",
                                        name=f"o{jb}_{kt}")
                        for t in range(IT):
                            nc.tensor.matmul(
                                ps_o[:mk, :],
                                qt[:, t, kt * PT:kt * PT + mk],
                                es[:, t, :],
                                start=(t == 0),
                                stop=(t == IT - 1),
                                skip_group_check=True,
                            )
                        if kt == CT - 1:
                            recip = rp.tile([1, JB], F32, tag="recip")
                            nc.vector.reciprocal(
                                recip[:], ps_o[ONE_ROW:ONE_ROW + 1, :])
                            bc = rp.tile([PT, JB], F32, tag="bc")
                            nc.gpsimd.partition_broadcast(bc[:], recip[:])
                        mo = LC if kt == CT - 1 else PT
                        osb = op.tile([PT, JB], F32, tag="osb",
                                      name=f"osb{jb}_{kt}")
                        nc.vector.tensor_mul(
                            out=osb[:mo, :], in0=ps_o[:mo, :],
                            in1=bc[:mo, :],
                        )
                        nc.sync.dma_start(
                            out[kt * PT:kt * PT + mo, js], osb[:mo, :])

            xtp.release()
            xfp.release()
            xqp.release()
            qtp.release()

    nc.compile()
    return nc


_CACHE = {}


def _get_program(P=4096, n_cores=8):
    key = (P, n_cores)
    if key not in _CACHE:
        _CACHE[key] = build(P, n_cores)
    return _CACHE[key]


def _run(inputs, trace=False, **kw):
    nc = _get_program()
    Xq = np.asarray(inputs["Xq"], dtype=np.float32)
    Xk = np.asarray(inputs["Xk"], dtype=np.float32)
    Wk = np.ascontiguousarray(np.asarray(inputs["Wk"], dtype=np.float32))
    bkv = np.ascontiguousarray(np.asarray(inputs["bk"], dtype=np.float32))
    Wv = np.ascontiguousarray(np.asarray(inputs["Wv"], dtype=np.float32))
    bvv = np.ascontiguousarray(np.asarray(inputs["bv"], dtype=np.float32))
    B = Xq.shape[0]
    in_maps = [
        {
            "Xq": np.ascontiguousarray(Xq[b]),
            "Xk": np.ascontiguousarray(Xk[b]),
            "Wk": Wk,
            "bk": bkv,
            "Wv": Wv,
            "bv": bvv,
        }
        for b in range(B)
    ]
    res = run_bass_kernel_spmd(nc, in_maps, list(range(B)), trace=trace, **kw)
    outs = np.stack([res.results[b]["out"] for b in range(B)], axis=0)
    return outs.astype(np.float32), res


def kernel(**inputs):
    outs, _ = _run(inputs)
    return outs


# revision 15
# speedup vs baseline: 1.1229x; 1.1229x over previous
"""CrossTransFormer attention kernel for 8x Trainium2 NeuronCores (Bass/Tile).

Problem (per batch b, B=8, C=773, P=4096):
    K = Wk @ Xk + bk            [C, P]
    V = Wv @ Xq + bv            [C, P]
    S[i, j] = sum_c K[c, i] * V[c, j] / sqrt(C)       (i, j over P)
    H = softmax(S, axis=i)
    out[k, j] = sum_i Xk[k, i] * H[i, j]              [C, P]

Sharding: data-parallel over batch, one batch per NeuronCore, no collectives.

Algebraic restructure (saves one full projection + all weight transposes):
    S = Xk^T (Wk^T Wv) Xq + u 1^T + 1 w^T   with u = Xk^T (Wk^T bv);
    the j-indexed w term is constant along the softmax axis i and cancels
    exactly -> dropped.
  GT = Wv^T Wk is computed on the PE with both weights in NATURAL layout,
  w1 = Wk^T bv rides along; both fold into the A-projection
  A = G Xq + w1 1^T.  The A-proj lhsT tiles are zero-padded to c1=896 so
  every projection chain emits full-128-partition PSUM tiles: the staged A
  is then zero-filled in its ragged rows FOR FREE, letting every S matmul
  run K=128.

QT = Xk^T is produced by the DMA XBAR transpose engine (14 batched
dma_start_transpose calls), entirely off the PE.  xk16 c-tile 6 carries an
all-ones row at partition 32, so the transpose plants an all-ones column at
qt col 800 for free; the ragged out-chain then lands softmax sums on PSUM
partition 32 (legal compute-engine base).  No plain SBUF->SBUF HWDGE DMAs
are issued anywhere (XBAR-transpose || SBUF->SBUF DMA is a known HW
deadlock): the w1 row is computed directly into PSUM partition 5 by giving
the bias column M=6 (cols 0..4 zero), and the xq6 ones row comes from a
memset-1.0 + partial overwrite.

Fused phase D (per j-block of 512), everything SBUF-resident:
  A-proj: 7 chains of 7 MMs -> ast[128, 7, 512] fp16 (no DRAM staging).
  S-phase: 32 i-tiles, 7-MM chains into triple-buffered PSUM, ACT exp
  (scale=1/sqrt(C)) into es[128, 32, 512] fp16.
  out-phase: 7 k-tile chains of 32 accumulating MMs; the ragged chain
  (5 data rows + softmax-sum row from the qt ones-column) runs FIRST so
  the reciprocal + partition-broadcast overlap the remaining chains;
  each chain is normalized (DVE) and DMA'd out as it finishes.
"""

import sys

sys.path.insert(0, "/opt/trn_rl_repo")

import numpy as np

import concourse.bacc as bacc
import concourse.mybir as mybir
import concourse.tile as tile
from concourse.bass_utils import run_bass_kernel_spmd

F32 = mybir.dt.float32
F16 = mybir.dt.float16

C = 773
PT = 128
CT = 7  # ceil(773 / 128) chunks of the channel dim
LC = C - (CT - 1) * PT  # 5 rows in the last chunk
JB = 512  # j-block width (one PSUM bank of fp32)
CW = CT * PT  # c1 padded to 896 for the zero-padded A-proj lhsT
QW = 6 * PT + 48  # qt width: 6 full c-tiles + 48-col XBAR tail block


def build(P=4096, n_cores=8):
    NJ = P // JB
    IT = P // PT
    SCALE = float(1.0 / np.sqrt(C))

    nc = bacc.Bacc("TRN2", target_bir_lowering=False, debug=False,
                   num_devices=n_cores)
    Xq = nc.dram_tensor("Xq", [C, P], F32, kind="ExternalInput")
    Xk = nc.dram_tensor("Xk", [C, P], F32, kind="ExternalInput")
    Wk = nc.dram_tensor("Wk", [C, C], F32, kind="ExternalInput")
    bk = nc.dram_tensor("bk", [C], F32, kind="ExternalInput")
    Wv = nc.dram_tensor("Wv", [C, C], F32, kind="ExternalInput")
    bv = nc.dram_tensor("bv", [C], F32, kind="ExternalInput")
    out = nc.dram_tensor("out", [C, P], F32, kind="ExternalOutput")
    del bk  # only enters via a softmax-invariant per-j term

    with tile.TileContext(nc) as tc:
        with tc.tile_pool(name="persist", bufs=1) as persist:
            # Xk fp16 resident, natural [c, p] layout: lhsT tiles for S.
            # Tile 6: rows 0..4 = ragged data, row 32 = all-ones (becomes
            # the qt ones-column via the XBAR transpose; contributes 0 to S
            # because ast tile-6 rows 5..127 are zero), rest zeros.
            xk16 = persist.tile([PT, CT, P], F16)
            # exp(S) for one j-block, [i-in-tile, it, j]
            es = persist.tile([PT, IT, JB], F16)
            # GT = Wv^T Wk [c2-part, ct2, c1] fp16, c1 zero-padded to 896
            g16 = persist.tile([PT, CT, CW], F16)
            # packed ragged lhsT: rows 0..4 = GT c2-ragged rows, row 5 = w1
            g6 = persist.tile([8, CW], F16)

            # PE warmup: dummy matmuls so the HAM clock-gate opens
            # (4/8 -> 8/8) while the first DMAs are in flight, and the
            # exp activation table loads before the main loop.  warm is
            # memset on DVE (gpsimd takes ~8us to boot).
            wsb = tc.alloc_tile_pool(name="wsb", bufs=1)
            warm = wsb.tile([PT, JB], F16)
            nc.vector.memset(warm[:, :], 0.0)
            with tc.tile_pool(name="pswarm", bufs=4, space="PSUM") as pswarm:
                for i in range(40):
                    wps = pswarm.tile([PT, JB], F32, tag="wps",
                                      name=f"wps{i}")
                    nc.tensor.matmul(wps[:, :], warm[:, :PT], warm[:, :],
                                     start=True, stop=True,
                                     skip_group_check=True)
                wexp = wsb.tile([1, 16], F32)
                nc.scalar.activation(wexp[:], wps[:1, :16],
                                     mybir.ActivationFunctionType.Exp,
                                     scale=1.0)
            wsb.release()

            # zero-pad fills on gpsimd (consumers run ~15us+, gpsimd boot
            # overlaps).  The xk16 tile-6 fills are issued on DVE but only
            # AFTER the W casts below, so the G-phase critical path is not
            # delayed; their first consumer is the jc-0 tail cast (~20us).
            nc.gpsimd.memset(g16[:, :, :], 0.0)
            nc.gpsimd.memset(g6[:, :], 0.0)

            # ---- Phase G: GT = Wv^T Wk and w1 = Wk^T bv on the PE ----
            with (
                tc.tile_pool(name="wload", bufs=1) as wload,
                tc.tile_pool(name="psg", bufs=4, space="PSUM") as psg,
            ):
                wk16 = wload.tile([PT, CT, C], F16, tag="wk16")
                wv16 = wload.tile([PT, CT, C], F16, tag="wv16")
                # bias columns, M=6 per o-tile: cols 0..4 zero, col 5 = bv
                # chunk -> the w1 chain emits w1 directly on PSUM row 5.
                bvcol = wload.tile([PT, CT, 6], F16, tag="bvcol")
                # batched W loads: 3 dma_starts per weight into an f32
                # staging ring, DVE-cast to fp16
                for Wsrc, dst in ((Wk, wk16), (Wv, wv16)):
                    for lo in (0, 3):
                        ws = wload.tile([PT, 3, C], F32, tag="wstage")
                        nc.sync.dma_start(
                            ws[:, :, :],
                            Wsrc[lo * PT:(lo + 3) * PT, :].rearrange(
                                "(ct p) c -> p ct c", p=PT),
                        )
                        nc.vector.tensor_copy(dst[:, lo:lo + 3, :],
                                              ws[:, :, :])
                    wt = wload.tile([8, C], F32, tag="wtail")
                    nc.sync.dma_start(wt[:LC, :], Wsrc[(CT - 1) * PT:C, :])
                    nc.vector.tensor_copy(dst[:LC, CT - 1, :], wt[:LC, :])
                # deferred DVE fills (after the W casts in DVE program
                # order).  xk16 tile 6: rows 0..4 = ragged data (cast in
                # phase B), row 32 = all-ones -> qt ones-column via XBAR.
                nc.vector.memset(xk16[:, CT - 1, :], 0.0)
                nc.vector.memset(xk16[32:33, CT - 1, :], 1.0)
                nc.vector.memset(bvcol[:, :, :], 0.0)
                # bv chunks into bvcol[:, ot, 5] on the gpsimd software
                # queue (DRAM->SBUF, cast f32->f16)
                for ot in range(CT - 1):
                    nc.gpsimd.dma_start(
                        bvcol[:, ot, 5:6], bv[ot * PT:(ot + 1) * PT, None])
                nc.gpsimd.dma_start(bvcol[:LC, CT - 1, 5:6],
                                    bv[(CT - 1) * PT:C, None])
                # GT tiles: [c2-tile, c1-chunk], contract over o (7 tiles)
                for ct2 in range(CT):
                    pc2 = PT if ct2 < CT - 1 else LC
                    for h, (j0, j1) in enumerate(((0, JB), (JB, C))):
                        ps = psg.tile([PT, JB], F32, tag="psg")
                        for ot in range(CT):
                            po = PT if ot < CT - 1 else LC
                            nc.tensor.matmul(
                                ps[:pc2, :j1 - j0],
                                wv16[:po, ot, ct2 * PT:ct2 * PT + pc2],
                                wk16[:po, ot, j0:j1],
                                start=(ot == 0),
                                stop=(ot == CT - 1),
                            )
                        # evacuate on the idle ACT engine: DVE is busy
                        # with W/Xk casts and would stall the G chains
                        nc.scalar.activation(
                            g16[:pc2, ct2, j0:j1], ps[:pc2, :j1 - j0],
                            mybir.ActivationFunctionType.Copy, scale=1.0)
                # w1 row: lhsT = bvcol (M=6, cols 0..4 zero) -> psum rows
                # 0..4 zero, row 5 = w1.  Copy rows 0..5 into g6 FIRST,
                # then overwrite rows 0..4 with the GT ragged rows (WAW
                # dep keeps the order).
                for h, (j0, j1) in enumerate(((0, JB), (JB, C))):
                    ps = psg.tile([8, JB], F32, tag="psw")
                    for ot in range(CT):
                        po = PT if ot < CT - 1 else LC
                        nc.tensor.matmul(
                            ps[:6, :j1 - j0],
                            bvcol[:po, ot, :],
                            wk16[:po, ot, j0:j1],
                            start=(ot == 0),
                            stop=(ot == CT - 1),
                        )
                    nc.scalar.activation(
                        g6[:6, j0:j1], ps[:6, :j1 - j0],
                        mybir.ActivationFunctionType.Copy, scale=1.0)
                nc.scalar.activation(
                    g6[:LC, :C], g16[:LC, CT - 1, :C],
                    mybir.ActivationFunctionType.Copy, scale=1.0)

            # QT pool reuses the space wload released.  qt[i, it, c]:
            # cols 0..767 from c-tiles 0..5, cols 768..815 from the 48-row
            # tail block (data rows 0..4 -> cols 768..772, ones row 32 ->
            # col 800, zeros elsewhere).
            qtp = tc.alloc_tile_pool(name="qtp", bufs=1)
            qt = qtp.tile([PT, IT, QW], F16)

            # pools that span phases B and D
            xqp = tc.alloc_tile_pool(name="xqp", bufs=2)
            xfp = tc.alloc_tile_pool(name="xfp", bufs=2)
            xtp = tc.alloc_tile_pool(name="xtp", bufs=2)

            def load_xq(jb):
                js = slice(jb * JB, (jb + 1) * JB)
                xq16 = xqp.tile([PT, CT, JB], F16, tag="xq16",
                                name=f"xq16_{jb}")
                for lo in (0, 3):
                    xf = xfp.tile([PT, 3, JB], F32, tag="xstage",
                                  name=f"xqf{jb}_{lo}")
                    nc.sync.dma_start(
                        xf[:, :, :],
                        Xq[lo * PT:(lo + 3) * PT, js].rearrange(
                            "(ct p) c -> p ct c", p=PT),
                    )
                    nc.vector.tensor_copy(xq16[:, lo:lo + 3, :],
                                          xf[:, :, :])
                xt = xtp.tile([8, JB], F32, tag="xtail", name=f"xqt{jb}")
                nc.sync.dma_start(xt[:LC, :], Xq[(CT - 1) * PT:C, js])
                nc.vector.tensor_copy(xq16[:LC, CT - 1, :], xt[:LC, :])
                # packed ragged rhs: memset 1.0 (row 5 stays ones, rows
                # 6..7 hit zero g6 rows), rows 0..4 overwritten with the
                # Xq c2-ragged rows.
                xq6 = xqp.tile([8, JB], F16, tag="xq6", name=f"xq6_{jb}")
                nc.vector.memset(xq6[:, :], 1.0)
                nc.vector.tensor_copy(xq6[:LC, :], xq16[:LC, CT - 1, :])
                return xq16, xq6

            # ---- Phase B: stream Xk -> resident fp16 (DMA + DVE only) --
            xq_next = load_xq(0)
            for jc in range(NJ):
                js = slice(jc * JB, (jc + 1) * JB)
                for lo in (0, 3):
                    xf = xfp.tile([PT, 3, JB], F32, tag="xstage",
                                  name=f"xkf{jc}_{lo}")
                    nc.sync.dma_start(
                        xf[:, :, :],
                        Xk[lo * PT:(lo + 3) * PT, js].rearrange(
                            "(ct p) c -> p ct c", p=PT),
                    )
                    nc.vector.tensor_copy(xk16[:, lo:lo + 3, js],
                                          xf[:, :, :])
                xt = xtp.tile([8, JB], F32, tag="xtail", name=f"xkt{jc}")
                nc.sync.dma_start(xt[:LC, :], Xk[(CT - 1) * PT:C, js])
                nc.vector.tensor_copy(xk16[:LC, CT - 1, js], xt[:LC, :])

            # ---- QT: batched XBAR transposes (off the PE entirely) ----
            # Two halves per c-tile so the first half can fire as soon as
            # jc 0..3 have landed.  out[p, t, f] = in[f, 128*t + p].
            for half in (0, 1):
                hp = slice(half * (P // 2), (half + 1) * (P // 2))
                ht = slice(half * (IT // 2), (half + 1) * (IT // 2))
                for ct in range(CT):
                    pc = PT if ct < CT - 1 else 48
                    nc.sync.dma_start_transpose(
                        qt[:, ht, ct * PT:ct * PT + pc],
                        xk16[:pc, ct, hp],
                    )

            # ---- Phase D: fused A-projection + attention main loop ----
            with (
                tc.tile_pool(name="astp", bufs=2) as astp,
                tc.tile_pool(name="op", bufs=2) as op,
                tc.tile_pool(name="rp", bufs=1) as rp,
                tc.tile_pool(name="psA", bufs=2, space="PSUM") as psA,
                tc.tile_pool(name="psS", bufs=3, space="PSUM") as psS,
                tc.tile_pool(name="psO", bufs=3, space="PSUM") as psO,
            ):
                for jb in range(NJ):
                    js = slice(jb * JB, (jb + 1) * JB)
                    xq16, xq6 = xq_next
                    if jb < NJ - 1:
                        xq_next = load_xq(jb + 1)

                    # A-proj: A[:, jblock] = G @ Xq + w1 (ragged K=6 MM
                    # carries both the c2 tail and the bias row); the
                    # zero-padded lhsT makes all 128 psum rows valid
                    ast = astp.tile([PT, CT, JB], F16, tag="ast",
                                    name=f"ast{jb}")
                    for ot in range(CT):
                        ps = psA.tile([PT, JB], F32, tag="a",
                                      name=f"a{jb}_{ot}")
                        for ct2 in range(CT - 1):
                            nc.tensor.matmul(
                                ps[:, :],
                                g16[:, ct2, ot * PT:(ot + 1) * PT],
                                xq16[:, ct2, :],
                                start=(ct2 == 0),
                                stop=False,
                                skip_group_check=True,
                            )
                        nc.tensor.matmul(
                            ps[:, :],
                            g6[:LC + 1, ot * PT:(ot + 1) * PT],
                            xq6[:LC + 1, :],
                            start=False,
                            stop=True,
                            skip_group_check=True,
                        )
                        nc.any.tensor_copy(ast[:, ot, :], ps[:, :])

                    # S-phase: 32 chains of 7 K=128 MMs, exp into es
                    for t in range(IT):
                        ts = slice(t * PT, (t + 1) * PT)
                        ps_s = psS.tile([PT, JB], F32, tag="s",
                                        name=f"s{jb}_{t}")
                        for ct in range(CT):
                            nc.tensor.matmul(
                                ps_s[:, :],
                                xk16[:, ct, ts],
                                ast[:, ct, :],
                                start=(ct == 0),
                                stop=(ct == CT - 1),
                                skip_group_check=True,
                            )
                        nc.scalar.activation(
                            es[:, t, :], ps_s[:],
                            mybir.ActivationFunctionType.Exp, scale=SCALE,
                        )

                    # out-phase: chain kt=0 FIRST (its early MMs only need
                    # the early es tiles, hiding the exp tail), then the
                    # ragged chain (data rows 768..772 + sums row from the
                    # qt ones column) so the reciprocal + broadcast still
                    # overlap the remaining chains.  kt=0's normalization
                    # is deferred until bc exists (PSUM bank held).
                    bc = None
                    held = None

                    def normalize(kt, ps_o):
                        mo = LC if kt == CT - 1 else PT
                        osb = op.tile([PT, JB], F32, tag="osb",
                                      name=f"osb{jb}_{kt}")
                        nc.vector.tensor_mul(
                            out=osb[:mo, :], in0=ps_o[:mo, :],
                            in1=bc[:mo, :],
                        )
                        nc.sync.dma_start(
                            out[kt * PT:kt * PT + mo, js], osb[:mo, :])

                    for kt in (0, CT - 1) + tuple(range(1, CT - 1)):
                        mk = 33 if kt == CT - 1 else PT
                        ps_o = psO.tile([PT, JB], F32, tag="o",
                                        name=f"o{jb}_{kt}")
                        for t in range(IT):
                            nc.tensor.matmul(
                                ps_o[:mk, :],
                                qt[:, t, kt * PT:kt * PT + mk],
                                es[:, t, :],
                                start=(t == 0),
                                stop=(t == IT - 1),
                                skip_group_check=True,
                            )
                        if kt == CT - 1:
                            recip = rp.tile([1, JB], F32, tag="recip")
                            nc.vector.reciprocal(recip[:], ps_o[32:33, :])
                            bc = rp.tile([PT, JB], F32, tag="bc")
                            nc.gpsimd.partition_broadcast(bc[:], recip[:])
                        if bc is None:
                            held = (kt, ps_o)
                            continue
                        normalize(kt, ps_o)
                        if held is not None:
                            normalize(*held)
                            held = None

            xtp.release()
            xfp.release()
            xqp.release()
            qtp.release()

    nc.compile()
    return nc


_CACHE = {}


def _get_program(P=4096, n_cores=8):
    key = (P, n_cores)
    if key not in _CACHE:
        _CACHE[key] = build(P, n_cores)
    return _CACHE[key]


def _run(inputs, trace=False, **kw):
    nc = _get_program()
    Xq = np.asarray(inputs["Xq"], dtype=np.float32)
    Xk = np.asarray(inputs["Xk"], dtype=np.float32)
    Wk = np.ascontiguousarray(np.asarray(inputs["Wk"], dtype=np.float32))
    bkv = np.ascontiguousarray(np.asarray(inputs["bk"], dtype=np.float32))
    Wv = np.ascontiguousarray(np.asarray(inputs["Wv"], dtype=np.float32))
    bvv = np.ascontiguousarray(np.asarray(inputs["bv"], dtype=np.float32))
    B = Xq.shape[0]
    in_maps = [
        {
            "Xq": np.ascontiguousarray(Xq[b]),
            "Xk": np.ascontiguousarray(Xk[b]),
            "Wk": Wk,
            "bk": bkv,
            "Wv": Wv,
            "bv": bvv,
        }
        for b in range(B)
    ]
    res = run_bass_kernel_spmd(nc, in_maps, list(range(B)), trace=trace, **kw)
    outs = np.stack([res.results[b]["out"] for b in range(B)], axis=0)
    return outs.astype(np.float32), res


def kernel(**inputs):
    outs, _ = _run(inputs)
    return outs


# revision 23
# speedup vs baseline: 1.1332x; 1.0091x over previous
"""CrossTransFormer attention kernel for 8x Trainium2 NeuronCores (Bass/Tile).

Problem (per batch b, B=8, C=773, P=4096):
    K = Wk @ Xk + bk            [C, P]
    V = Wv @ Xq + bv            [C, P]
    S[i, j] = sum_c K[c, i] * V[c, j] / sqrt(C)       (i, j over P)
    H = softmax(S, axis=i)
    out[k, j] = sum_i Xk[k, i] * H[i, j]              [C, P]

Sharding: data-parallel over batch, one batch per NeuronCore, no collectives.

Algebraic restructure (saves one full projection + all weight transposes):
    S = Xk^T (Wk^T Wv) Xq + u 1^T + 1 w^T   with u = Xk^T (Wk^T bv);
    the j-indexed w term is constant along the softmax axis i and cancels
    exactly -> dropped.
  GT = Wv^T Wk is computed on the PE with both weights in NATURAL layout,
  w1 = Wk^T bv rides along; both fold into the A-projection
  A = G Xq + w1 1^T.  The A-proj lhsT tiles are zero-padded to c1=896 so
  every projection chain emits full-128-partition PSUM tiles: the staged A
  is then zero-filled in its ragged rows FOR FREE, letting every S matmul
  run K=128.

QT = Xk^T is produced by the DMA XBAR transpose engine (14 batched
dma_start_transpose calls), entirely off the PE.  xk16 c-tile 6 carries an
all-ones row at partition 32, so the transpose plants an all-ones column at
qt col 800 for free; the ragged out-chain then lands softmax sums on PSUM
partition 32 (legal compute-engine base).  No plain SBUF->SBUF HWDGE DMAs
are issued anywhere (XBAR-transpose || SBUF->SBUF DMA is a known HW
deadlock): the w1 row is computed directly into PSUM partition 5 by giving
the bias column M=6 (cols 0..4 zero), and the xq6 ones row comes from a
memset-1.0 + partial overwrite.

Fused phase D (per j-block of 512), everything SBUF-resident:
  A-proj: 7 chains of 7 MMs -> ast[128, 7, 512] fp16 (no DRAM staging).
  S-phase: 32 i-tiles, 7-MM chains into triple-buffered PSUM, ACT exp
  (scale=1/sqrt(C)) into es[128, 32, 512] fp16.
  out-phase: 7 k-tile chains of 32 accumulating MMs; the ragged chain
  (5 data rows + softmax-sum row from the qt ones-column) runs FIRST so
  the reciprocal + partition-broadcast overlap the remaining chains;
  each chain is normalized (DVE) and DMA'd out as it finishes.
"""

import sys

sys.path.insert(0, "/opt/trn_rl_repo")

import numpy as np

import concourse.bacc as bacc
import concourse.mybir as mybir
import concourse.tile as tile
from concourse.bass_utils import run_bass_kernel_spmd

F32 = mybir.dt.float32
F16 = mybir.dt.float16

C = 773
PT = 128
CT = 7  # ceil(773 / 128) chunks of the channel dim
LC = C - (CT - 1) * PT  # 5 rows in the last chunk
JB = 512  # j-block width (one PSUM bank of fp32)
CW = CT * PT  # c1 padded to 896 for the zero-padded A-proj lhsT
QW = 6 * PT + 48  # qt width: 6 full c-tiles + 48-col XBAR tail block


def build(P=4096, n_cores=8):
    NJ = P // JB
    IT = P // PT
    SCALE = float(1.0 / np.sqrt(C))

    nc = bacc.Bacc("TRN2", target_bir_lowering=False, debug=False,
                   num_devices=n_cores)
    Xq = nc.dram_tensor("Xq", [C, P], F32, kind="ExternalInput")
    Xk = nc.dram_tensor("Xk", [C, P], F32, kind="ExternalInput")
    Wk = nc.dram_tensor("Wk", [C, C], F32, kind="ExternalInput")
    bk = nc.dram_tensor("bk", [C], F32, kind="ExternalInput")
    Wv = nc.dram_tensor("Wv", [C, C], F32, kind="ExternalInput")
    bv = nc.dram_tensor("bv", [C], F32, kind="ExternalInput")
    out = nc.dram_tensor("out", [C, P], F32, kind="ExternalOutput")
    del bk  # only enters via a softmax-invariant per-j term

    with tile.TileContext(nc) as tc:
        with tc.tile_pool(name="persist", bufs=1) as persist:
            # Xk fp16 resident, natural [c, p] layout: lhsT tiles for S.
            # Tile 6: rows 0..4 = ragged data, row 32 = all-ones (becomes
            # the qt ones-column via the XBAR transpose; contributes 0 to S
            # because ast tile-6 rows 5..127 are zero), rest zeros.
            xk16 = persist.tile([PT, CT, P], F16)
            # exp(S) for one j-block, [i-in-tile, it, j]
            es = persist.tile([PT, IT, JB], F16)
            # GT = Wv^T Wk [c2-part, ct2, c1] fp16, c1 zero-padded to 896
            g16 = persist.tile([PT, CT, CW], F16)
            # packed ragged lhsT: rows 0..4 = GT c2-ragged rows, row 5 = w1
            g6 = persist.tile([8, CW], F16)

            # PE warmup: dummy matmuls so the HAM clock-gate opens
            # (4/8 -> 8/8) while the first DMAs are in flight, and the
            # exp activation table loads before the main loop.  warm is
            # memset on DVE (gpsimd takes ~8us to boot).
            wsb = tc.alloc_tile_pool(name="wsb", bufs=1)
            warm = wsb.tile([PT, JB], F16)
            nc.vector.memset(warm[:, :], 0.0)
            with tc.tile_pool(name="pswarm", bufs=4, space="PSUM") as pswarm:
                for i in range(52):
                    wps = pswarm.tile([PT, JB], F32, tag="wps",
                                      name=f"wps{i}")
                    nc.tensor.matmul(wps[:, :], warm[:, :PT], warm[:, :],
                                     start=True, stop=True,
                                     skip_group_check=True)
                wexp = wsb.tile([1, 16], F32)
                nc.scalar.activation(wexp[:], wps[:1, :16],
                                     mybir.ActivationFunctionType.Exp,
                                     scale=1.0)
            wsb.release()

            # zero-pad fills on gpsimd (consumers run ~15us+, gpsimd boot
            # overlaps).  The xk16 tile-6 fills are issued on DVE but only
            # AFTER the W casts below, so the G-phase critical path is not
            # delayed; their first consumer is the jc-0 tail cast (~20us).
            nc.gpsimd.memset(g16[:, :, :], 0.0)
            nc.gpsimd.memset(g6[:, :], 0.0)

            # ---- Phase G: GT = Wv^T Wk and w1 = Wk^T bv on the PE ----
            with (
                tc.tile_pool(name="wstg", bufs=3) as wstg,
                tc.tile_pool(name="wload", bufs=1) as wload,
                tc.tile_pool(name="psg", bufs=4, space="PSUM") as psg,
            ):
                wk16 = wload.tile([PT, CT, C], F16, tag="wk16")
                wv16 = wload.tile([PT, CT, C], F16, tag="wv16")
                # bias columns, M=6 per o-tile: cols 0..4 zero, col 5 = bv
                # chunk -> the w1 chain emits w1 directly on PSUM row 5.
                bvcol = wload.tile([PT, CT, 6], F16, tag="bvcol")
                # batched W loads: 3 dma_starts per weight into an f32
                # staging ring, DVE-cast to fp16
                for Wsrc, dst in ((Wk, wk16), (Wv, wv16)):
                    for lo in (0, 3):
                        ws = wstg.tile([PT, 3, C], F32, tag="wstage")
                        nc.sync.dma_start(
                            ws[:, :, :],
                            Wsrc[lo * PT:(lo + 3) * PT, :].rearrange(
                                "(ct p) c -> p ct c", p=PT),
                        )
                        nc.vector.tensor_copy(dst[:, lo:lo + 3, :],
                                              ws[:, :, :])
                    wt = wstg.tile([8, C], F32, tag="wtail")
                    nc.sync.dma_start(wt[:LC, :], Wsrc[(CT - 1) * PT:C, :])
                    nc.vector.tensor_copy(dst[:LC, CT - 1, :], wt[:LC, :])
                # deferred DVE fills (after the W casts in DVE program
                # order).  xk16 tile 6: rows 0..4 = ragged data (cast in
                # phase B), row 32 = all-ones -> qt ones-column via XBAR.
                nc.vector.memset(xk16[:, CT - 1, :], 0.0)
                nc.vector.memset(xk16[32:33, CT - 1, :], 1.0)
                nc.vector.memset(bvcol[:, :, :], 0.0)
                # bv chunks into bvcol[:, ot, 5] on the gpsimd software
                # queue (DRAM->SBUF, cast f32->f16)
                for ot in range(CT - 1):
                    nc.gpsimd.dma_start(
                        bvcol[:, ot, 5:6], bv[ot * PT:(ot + 1) * PT, None])
                nc.gpsimd.dma_start(bvcol[:LC, CT - 1, 5:6],
                                    bv[(CT - 1) * PT:C, None])
                # GT tiles: [c2-tile, c1-chunk], contract over o (7 tiles)
                for ct2 in range(CT):
                    pc2 = PT if ct2 < CT - 1 else LC
                    for h, (j0, j1) in enumerate(((0, JB), (JB, C))):
                        ps = psg.tile([PT, JB], F32, tag="psg")
                        for ot in range(CT):
                            po = PT if ot < CT - 1 else LC
                            nc.tensor.matmul(
                                ps[:pc2, :j1 - j0],
                                wv16[:po, ot, ct2 * PT:ct2 * PT + pc2],
                                wk16[:po, ot, j0:j1],
                                start=(ot == 0),
                                stop=(ot == CT - 1),
                            )
                        # evacuate on the idle ACT engine: DVE is busy
                        # with W/Xk casts and would stall the G chains
                        nc.scalar.activation(
                            g16[:pc2, ct2, j0:j1], ps[:pc2, :j1 - j0],
                            mybir.ActivationFunctionType.Copy, scale=1.0)
                # w1 row: lhsT = bvcol (M=6, cols 0..4 zero) -> psum rows
                # 0..4 zero, row 5 = w1.  Copy rows 0..5 into g6 FIRST,
                # then overwrite rows 0..4 with the GT ragged rows (WAW
                # dep keeps the order).
                for h, (j0, j1) in enumerate(((0, JB), (JB, C))):
                    ps = psg.tile([8, JB], F32, tag="psw")
                    for ot in range(CT):
                        po = PT if ot < CT - 1 else LC
                        nc.tensor.matmul(
                            ps[:6, :j1 - j0],
                            bvcol[:po, ot, :],
                            wk16[:po, ot, j0:j1],
                            start=(ot == 0),
                            stop=(ot == CT - 1),
                        )
                    nc.scalar.activation(
                        g6[:6, j0:j1], ps[:6, :j1 - j0],
                        mybir.ActivationFunctionType.Copy, scale=1.0)
                nc.scalar.activation(
                    g6[:LC, :C], g16[:LC, CT - 1, :C],
                    mybir.ActivationFunctionType.Copy, scale=1.0)

            # QT pool reuses the space wload released.  qt[i, it, c]:
            # cols 0..767 from c-tiles 0..5, cols 768..815 from the 48-row
            # tail block (data rows 0..4 -> cols 768..772, ones row 32 ->
            # col 800, zeros elsewhere).
            qtp = tc.alloc_tile_pool(name="qtp", bufs=1)
            qt = qtp.tile([PT, IT, QW], F16)

            # pools that span phases B and D.  xfp is a deep per-chunk
            # staging ring: input DMA throughput scales with the number of
            # in-flight dma_starts (~20 GB/s per queue), so 7 concurrent
            # 256 KB chunk loads pull ~2x the aggregate bandwidth of the
            # 2-deep batched scheme.
            xqp = tc.alloc_tile_pool(name="xqp", bufs=2)
            xfp = tc.alloc_tile_pool(name="xfp", bufs=7)
            xtp = tc.alloc_tile_pool(name="xtp", bufs=2)

            def load_xq(jb):
                js = slice(jb * JB, (jb + 1) * JB)
                xq16 = xqp.tile([PT, CT, JB], F16, tag="xq16",
                                name=f"xq16_{jb}")
                for ct in range(CT - 1):
                    xf = xfp.tile([PT, JB], F32, tag="xstage",
                                  name=f"xqf{jb}_{ct}")
                    nc.sync.dma_start(
                        xf[:, :], Xq[ct * PT:(ct + 1) * PT, js])
                    nc.vector.tensor_copy(xq16[:, ct, :], xf[:, :])
                xt = xtp.tile([8, JB], F32, tag="xtail", name=f"xqt{jb}")
                nc.sync.dma_start(xt[:LC, :], Xq[(CT - 1) * PT:C, js])
                nc.vector.tensor_copy(xq16[:LC, CT - 1, :], xt[:LC, :])
                # packed ragged rhs: memset 1.0 (row 5 stays ones, rows
                # 6..7 hit zero g6 rows), rows 0..4 overwritten with the
                # Xq c2-ragged rows.
                xq6 = xqp.tile([8, JB], F16, tag="xq6", name=f"xq6_{jb}")
                nc.vector.memset(xq6[:, :], 1.0)
                nc.vector.tensor_copy(xq6[:LC, :], xq16[:LC, CT - 1, :])
                return xq16, xq6

            # ---- Phase B: stream Xk -> resident fp16 (DMA + DVE only);
            # xq block 0 is interleaved after jc 0 so the Xk stream (the
            # long pole for S0) starts first. ----
            xq_next = None
            for jc in range(NJ):
                js = slice(jc * JB, (jc + 1) * JB)
                for ct in range(CT - 1):
                    xf = xfp.tile([PT, JB], F32, tag="xstage",
                                  name=f"xkf{jc}_{ct}")
                    nc.sync.dma_start(
                        xf[:, :], Xk[ct * PT:(ct + 1) * PT, js])
                    nc.vector.tensor_copy(xk16[:, ct, js], xf[:, :])
                xt = xtp.tile([8, JB], F32, tag="xtail", name=f"xkt{jc}")
                nc.sync.dma_start(xt[:LC, :], Xk[(CT - 1) * PT:C, js])
                nc.vector.tensor_copy(xk16[:LC, CT - 1, js], xt[:LC, :])
                if jc == 0:
                    xq_next = load_xq(0)

            # ---- QT: batched XBAR transposes (off the PE entirely) ----
            # Two halves per c-tile so the first half can fire as soon as
            # jc 0..3 have landed.  out[p, t, f] = in[f, 128*t + p].
            for half in (0, 1):
                hp = slice(half * (P // 2), (half + 1) * (P // 2))
                ht = slice(half * (IT // 2), (half + 1) * (IT // 2))
                for ct in range(CT):
                    pc = PT if ct < CT - 1 else 48
                    nc.sync.dma_start_transpose(
                        qt[:, ht, ct * PT:ct * PT + pc],
                        xk16[:pc, ct, hp],
                    )

            # ---- Phase D: fused A-projection + attention main loop ----
            with (
                tc.tile_pool(name="astp", bufs=2) as astp,
                tc.tile_pool(name="op", bufs=2) as op,
                tc.tile_pool(name="rp", bufs=1) as rp,
                tc.tile_pool(name="psA", bufs=2, space="PSUM") as psA,
                tc.tile_pool(name="psS", bufs=3, space="PSUM") as psS,
                tc.tile_pool(name="psO", bufs=3, space="PSUM") as psO,
            ):
                for jb in range(NJ):
                    js = slice(jb * JB, (jb + 1) * JB)
                    xq16, xq6 = xq_next
                    if jb < NJ - 1:
                        xq_next = load_xq(jb + 1)

                    # A-proj: A[:, jblock] = G @ Xq + w1 (ragged K=6 MM
                    # carries both the c2 tail and the bias row); the
                    # zero-padded lhsT makes all 128 psum rows valid
                    ast = astp.tile([PT, CT, JB], F16, tag="ast",
                                    name=f"ast{jb}")
                    for ot in range(CT):
                        ps = psA.tile([PT, JB], F32, tag="a",
                                      name=f"a{jb}_{ot}")
                        for ct2 in range(CT - 1):
                            nc.tensor.matmul(
                                ps[:, :],
                                g16[:, ct2, ot * PT:(ot + 1) * PT],
                                xq16[:, ct2, :],
                                start=(ct2 == 0),
                                stop=False,
                                skip_group_check=True,
                            )
                        nc.tensor.matmul(
                            ps[:, :],
                            g6[:LC + 1, ot * PT:(ot + 1) * PT],
                            xq6[:LC + 1, :],
                            start=False,
                            stop=True,
                            skip_group_check=True,
                        )
                        nc.any.tensor_copy(ast[:, ot, :], ps[:, :])

                    # S-phase: 32 chains of 7 K=128 MMs, exp into es
                    for t in range(IT):
                        ts = slice(t * PT, (t + 1) * PT)
                        ps_s = psS.tile([PT, JB], F32, tag="s",
                                        name=f"s{jb}_{t}")
                        for ct in range(CT):
                            nc.tensor.matmul(
                                ps_s[:, :],
                                xk16[:, ct, ts],
                                ast[:, ct, :],
                                start=(ct == 0),
                                stop=(ct == CT - 1),
                                skip_group_check=True,
                            )
                        nc.scalar.activation(
                            es[:, t, :], ps_s[:],
                            mybir.ActivationFunctionType.Exp, scale=SCALE,
                        )

                    # out-phase: chain kt=0 FIRST (its early MMs only need
                    # the early es tiles, hiding the exp tail), then the
                    # ragged chain (data rows 768..772 + sums row from the
                    # qt ones column) so the reciprocal + broadcast still
                    # overlap the remaining chains.  kt=0's normalization
                    # is deferred until bc exists (PSUM bank held).
                    bc = None
                    held = None

                    def normalize(kt, ps_o):
                        mo = LC if kt == CT - 1 else PT
                        osb = op.tile([PT, JB], F32, tag="osb",
                                      name=f"osb{jb}_{kt}")
                        nc.vector.tensor_mul(
                            out=osb[:mo, :], in0=ps_o[:mo, :],
                            in1=bc[:mo, :],
                        )
                        nc.sync.dma_start(
                            out[kt * PT:kt * PT + mo, js], osb[:mo, :])

                    for kt in (0, CT - 1) + tuple(range(1, CT - 1)):
                        mk = 33 if kt == CT - 1 else PT
                        ps_o = psO.tile([PT, JB], F32, tag="o",
                                        name=f"o{jb}_{kt}")
                        for t in range(IT):
                            nc.tensor.matmul(
                                ps_o[:mk, :],
                                qt[:, t, kt * PT:kt * PT + mk],
                                es[:, t, :],
                                start=(t == 0),
                                stop=(t == IT - 1),
                                skip_group_check=True,
                            )
                        if kt == CT - 1:
                            # fp16 1/den costs ~5e-4 relative error on the
                            # output, far under the 2e-2 gate; halves SBUF
                            recip = rp.tile([1, JB], F16, tag="recip")
                            with nc.allow_low_precision(
                                    reason="fp16 softmax denom recip"):
                                nc.vector.reciprocal(
                                    recip[:], ps_o[32:33, :])
                            bc = rp.tile([PT, JB], F16, tag="bc")
                            nc.gpsimd.partition_broadcast(bc[:], recip[:])
                        if bc is None:
                            held = (kt, ps_o)
                            continue
                        normalize(kt, ps_o)
                        if held is not None:
                            normalize(*held)
                            held = None

            xtp.release()
            xfp.release()
            xqp.release()
            qtp.release()

    nc.compile()
    return nc


_CACHE = {}


def _get_program(P=4096, n_cores=8):
    key = (P, n_cores)
    if key not in _CACHE:
        _CACHE[key] = build(P, n_cores)
    return _CACHE[key]


def _run(inputs, trace=False, **kw):
    nc = _get_program()
    Xq = np.asarray(inputs["Xq"], dtype=np.float32)
    Xk = np.asarray(inputs["Xk"], dtype=np.float32)
    Wk = np.ascontiguousarray(np.asarray(inputs["Wk"], dtype=np.float32))
    bkv = np.ascontiguousarray(np.asarray(inputs["bk"], dtype=np.float32))
    Wv = np.ascontiguousarray(np.asarray(inputs["Wv"], dtype=np.float32))
    bvv = np.ascontiguousarray(np.asarray(inputs["bv"], dtype=np.float32))
    B = Xq.shape[0]
    in_maps = [
        {
            "Xq": np.ascontiguousarray(Xq[b]),
            "Xk": np.ascontiguousarray(Xk[b]),
            "Wk": Wk,
            "bk": bkv,
            "Wv": Wv,
            "bv": bvv,
        }
        for b in range(B)
    ]
    res = run_bass_kernel_spmd(nc, in_maps, list(range(B)), trace=trace, **kw)
    outs = np.stack([res.results[b]["out"] for b in range(B)], axis=0)
    return outs.astype(np.float32), res


def kernel(**inputs):
    outs, _ = _run(inputs)
    return outs


# revision 27
# speedup vs baseline: 1.1462x; 1.0115x over previous
"""CrossTransFormer attention kernel for 8x Trainium2 NeuronCores (Bass/Tile).

Problem (per batch b, B=8, C=773, P=4096):
    K = Wk @ Xk + bk            [C, P]
    V = Wv @ Xq + bv            [C, P]
    S[i, j] = sum_c K[c, i] * V[c, j] / sqrt(C)       (i, j over P)
    H = softmax(S, axis=i)
    out[k, j] = sum_i Xk[k, i] * H[i, j]              [C, P]

Sharding: data-parallel over batch, one batch per NeuronCore, no collectives.

Algebraic restructure (saves one full projection + all weight transposes):
    S = Xk^T (Wk^T Wv) Xq + u 1^T + 1 w^T   with u = Xk^T (Wk^T bv);
    the j-indexed w term is constant along the softmax axis i and cancels
    exactly -> dropped.
  GT = Wv^T Wk is computed on the PE with both weights in NATURAL layout,
  w1 = Wk^T bv rides along; both fold into the A-projection
  A = G Xq + w1 1^T.  The A-proj lhsT tiles are zero-padded to c1=896 so
  every projection chain emits full-128-partition PSUM tiles: the staged A
  is then zero-filled in its ragged rows FOR FREE, letting every S matmul
  run K=128.

QT = Xk^T is produced by the DMA XBAR transpose engine (14 batched
dma_start_transpose calls), entirely off the PE.  xk16 c-tile 6 carries an
all-ones row at partition 32, so the transpose plants an all-ones column at
qt col 800 for free; the ragged out-chain then lands softmax sums on PSUM
partition 32 (legal compute-engine base).  No plain SBUF->SBUF HWDGE DMAs
are issued anywhere (XBAR-transpose || SBUF->SBUF DMA is a known HW
deadlock): the w1 row is computed directly into PSUM partition 5 by giving
the bias column M=6 (cols 0..4 zero), and the xq6 ones row comes from a
memset-1.0 + partial overwrite.

Fused phase D (per j-block of 512), everything SBUF-resident:
  A-proj: 7 chains of 7 MMs -> ast[128, 7, 512] fp16 (no DRAM staging).
  S-phase: 32 i-tiles, 7-MM chains into triple-buffered PSUM, ACT exp
  (scale=1/sqrt(C)) into es[128, 32, 512] fp16.
  out-phase: 7 k-tile chains of 32 accumulating MMs; the ragged chain
  (5 data rows + softmax-sum row from the qt ones-column) runs FIRST so
  the reciprocal + partition-broadcast overlap the remaining chains;
  each chain is normalized (DVE) and DMA'd out as it finishes.
"""

import sys

sys.path.insert(0, "/opt/trn_rl_repo")

import numpy as np

import concourse.bacc as bacc
import concourse.mybir as mybir
import concourse.tile as tile
from concourse.bass_utils import run_bass_kernel_spmd

F32 = mybir.dt.float32
F16 = mybir.dt.float16

C = 773
PT = 128
CT = 7  # ceil(773 / 128) chunks of the channel dim
LC = C - (CT - 1) * PT  # 5 rows in the last chunk
JB = 512  # j-block width (one PSUM bank of fp32)
CW = CT * PT  # c1 padded to 896 for the zero-padded A-proj lhsT
QW = 6 * PT + 48  # qt width: 6 full c-tiles + 48-col XBAR tail block


def build(P=4096, n_cores=8):
    NJ = P // JB
    IT = P // PT
    SCALE = float(1.0 / np.sqrt(C))

    nc = bacc.Bacc("TRN2", target_bir_lowering=False, debug=False,
                   num_devices=n_cores)
    Xq = nc.dram_tensor("Xq", [C, P], F32, kind="ExternalInput")
    Xk = nc.dram_tensor("Xk", [C, P], F32, kind="ExternalInput")
    Wk = nc.dram_tensor("Wk", [C, C], F32, kind="ExternalInput")
    bk = nc.dram_tensor("bk", [C], F32, kind="ExternalInput")
    Wv = nc.dram_tensor("Wv", [C, C], F32, kind="ExternalInput")
    bv = nc.dram_tensor("bv", [C], F32, kind="ExternalInput")
    out = nc.dram_tensor("out", [C, P], F32, kind="ExternalOutput")
    del bk  # only enters via a softmax-invariant per-j term

    with tile.TileContext(nc) as tc:
        with tc.tile_pool(name="persist", bufs=1) as persist:
            # Xk fp16 resident, natural [c, p] layout: lhsT tiles for S.
            # Tile 6: rows 0..4 = ragged data, row 32 = all-ones (becomes
            # the qt ones-column via the XBAR transpose; contributes 0 to S
            # because ast tile-6 rows 5..127 are zero), rest zeros.
            xk16 = persist.tile([PT, CT, P], F16)
            # exp(S) for one j-block, [i-in-tile, it, j]
            es = persist.tile([PT, IT, JB], F16)
            # GT = Wv^T Wk [c2-part, ct2, c1] fp16, c1 zero-padded to 896
            g16 = persist.tile([PT, CT, CW], F16)
            # packed ragged lhsT: rows 0..4 = GT c2-ragged rows, row 5 = w1
            g6 = persist.tile([8, CW], F16)

            # PE warmup: dummy matmuls so the HAM clock-gate opens
            # (4/8 -> 8/8) while the first DMAs are in flight, and the
            # exp activation table loads before the main loop.  warm is
            # memset on DVE (gpsimd takes ~8us to boot).
            wsb = tc.alloc_tile_pool(name="wsb", bufs=1)
            warm = wsb.tile([PT, JB], F16)
            nc.vector.memset(warm[:, :], 0.0)
            with tc.tile_pool(name="pswarm", bufs=4, space="PSUM") as pswarm:
                for i in range(52):
                    wps = pswarm.tile([PT, JB], F32, tag="wps",
                                      name=f"wps{i}")
                    nc.tensor.matmul(wps[:, :], warm[:, :PT], warm[:, :],
                                     start=True, stop=True,
                                     skip_group_check=True)
                wexp = wsb.tile([1, 16], F32)
                nc.scalar.activation(wexp[:], wps[:1, :16],
                                     mybir.ActivationFunctionType.Exp,
                                     scale=1.0)
            wsb.release()

            # zero-pad fills on gpsimd (consumers run ~15us+, gpsimd boot
            # overlaps).  The xk16 tile-6 fills are issued on DVE but only
            # AFTER the W casts below, so the G-phase critical path is not
            # delayed; their first consumer is the jc-0 tail cast (~20us).
            nc.gpsimd.memset(g16[:, :, :], 0.0)
            nc.gpsimd.memset(g6[:, :], 0.0)

            # ---- Phase G: GT = Wv^T Wk and w1 = Wk^T bv on the PE ----
            with (
                tc.tile_pool(name="wstg", bufs=6) as wstg,
                tc.tile_pool(name="wtlp", bufs=2) as wtlp,
                tc.tile_pool(name="wload", bufs=1) as wload,
                tc.tile_pool(name="psg", bufs=4, space="PSUM") as psg,
            ):
                wk16 = wload.tile([PT, CT, C], F16, tag="wk16")
                wv16 = wload.tile([PT, CT, C], F16, tag="wv16")
                # bias columns, M=6 per o-tile: cols 0..4 zero, col 5 = bv
                # chunk -> the w1 chain emits w1 directly on PSUM row 5.
                bvcol = wload.tile([PT, CT, 6], F16, tag="bvcol")
                # per-chunk W loads through a 6-deep ring: 12 concurrent
                # DMAs pull ~250 GB/s aggregate (one queue sustains only
                # ~20 GB/s), landing W in ~12us so phase G starts early.
                for Wsrc, dst in ((Wk, wk16), (Wv, wv16)):
                    for ct in range(CT - 1):
                        ws = wstg.tile([PT, C], F32, tag="wstage")
                        nc.sync.dma_start(
                            ws[:, :], Wsrc[ct * PT:(ct + 1) * PT, :])
                        nc.vector.tensor_copy(dst[:, ct, :], ws[:, :])
                    wt = wtlp.tile([8, C], F32, tag="wtail")
                    nc.sync.dma_start(wt[:LC, :], Wsrc[(CT - 1) * PT:C, :])
                    nc.vector.tensor_copy(dst[:LC, CT - 1, :], wt[:LC, :])
                # deferred DVE fills (after the W casts in DVE program
                # order).  xk16 tile 6: rows 0..4 = ragged data (cast in
                # phase B), row 32 = all-ones -> qt ones-column via XBAR.
                nc.vector.memset(xk16[:, CT - 1, :], 0.0)
                nc.vector.memset(xk16[32:33, CT - 1, :], 1.0)
                nc.vector.memset(bvcol[:, :, :], 0.0)
                # bv chunks into bvcol[:, ot, 5] on the gpsimd software
                # queue (DRAM->SBUF, cast f32->f16)
                for ot in range(CT - 1):
                    nc.gpsimd.dma_start(
                        bvcol[:, ot, 5:6], bv[ot * PT:(ot + 1) * PT, None])
                nc.gpsimd.dma_start(bvcol[:LC, CT - 1, 5:6],
                                    bv[(CT - 1) * PT:C, None])
                # GT tiles: [c2-tile, c1-chunk], contract over o (7 tiles)
                for ct2 in range(CT):
                    pc2 = PT if ct2 < CT - 1 else LC
                    for h, (j0, j1) in enumerate(((0, JB), (JB, C))):
                        ps = psg.tile([PT, JB], F32, tag="psg")
                        for ot in range(CT):
                            po = PT if ot < CT - 1 else LC
                            nc.tensor.matmul(
                                ps[:pc2, :j1 - j0],
                                wv16[:po, ot, ct2 * PT:ct2 * PT + pc2],
                                wk16[:po, ot, j0:j1],
                                start=(ot == 0),
                                stop=(ot == CT - 1),
                            )
                        # evacuate on the idle ACT engine: DVE is busy
                        # with W/Xk casts and would stall the G chains
                        nc.scalar.activation(
                            g16[:pc2, ct2, j0:j1], ps[:pc2, :j1 - j0],
                            mybir.ActivationFunctionType.Copy, scale=1.0)
                # w1 row: lhsT = bvcol (M=6, cols 0..4 zero) -> psum rows
                # 0..4 zero, row 5 = w1.  Copy rows 0..5 into g6 FIRST,
                # then overwrite rows 0..4 with the GT ragged rows (WAW
                # dep keeps the order).
                for h, (j0, j1) in enumerate(((0, JB), (JB, C))):
                    ps = psg.tile([8, JB], F32, tag="psw")
                    for ot in range(CT):
                        po = PT if ot < CT - 1 else LC
                        nc.tensor.matmul(
                            ps[:6, :j1 - j0],
                            bvcol[:po, ot, :],
                            wk16[:po, ot, j0:j1],
                            start=(ot == 0),
                            stop=(ot == CT - 1),
                        )
                    nc.scalar.activation(
                        g6[:6, j0:j1], ps[:6, :j1 - j0],
                        mybir.ActivationFunctionType.Copy, scale=1.0)
                nc.scalar.activation(
                    g6[:LC, :C], g16[:LC, CT - 1, :C],
                    mybir.ActivationFunctionType.Copy, scale=1.0)

            # QT pool reuses the space wload released.  qt[i, it, c]:
            # cols 0..767 from c-tiles 0..5, cols 768..815 from the 48-row
            # tail block (data rows 0..4 -> cols 768..772, ones row 32 ->
            # col 800, zeros elsewhere).
            qtp = tc.alloc_tile_pool(name="qtp", bufs=1)
            qt = qtp.tile([PT, IT, QW], F16)

            # pools that span phases B and D.  xfp is a deep per-chunk
            # staging ring: input DMA throughput scales with the number of
            # in-flight dma_starts (~20 GB/s per queue), so 7 concurrent
            # 256 KB chunk loads pull ~2x the aggregate bandwidth of the
            # 2-deep batched scheme.
            xqp = tc.alloc_tile_pool(name="xqp", bufs=2)
            xfp = tc.alloc_tile_pool(name="xfp", bufs=7)
            xtp = tc.alloc_tile_pool(name="xtp", bufs=2)

            def load_xq(jb):
                js = slice(jb * JB, (jb + 1) * JB)
                xq16 = xqp.tile([PT, CT, JB], F16, tag="xq16",
                                name=f"xq16_{jb}")
                for ct in range(CT - 1):
                    xf = xfp.tile([PT, JB], F32, tag="xstage",
                                  name=f"xqf{jb}_{ct}")
                    nc.sync.dma_start(
                        xf[:, :], Xq[ct * PT:(ct + 1) * PT, js])
                    nc.vector.tensor_copy(xq16[:, ct, :], xf[:, :])
                xt = xtp.tile([8, JB], F32, tag="xtail", name=f"xqt{jb}")
                nc.sync.dma_start(xt[:LC, :], Xq[(CT - 1) * PT:C, js])
                nc.vector.tensor_copy(xq16[:LC, CT - 1, :], xt[:LC, :])
                # packed ragged rhs: memset 1.0 (row 5 stays ones, rows
                # 6..7 hit zero g6 rows), rows 0..4 overwritten with the
                # Xq c2-ragged rows.
                xq6 = xqp.tile([8, JB], F16, tag="xq6", name=f"xq6_{jb}")
                nc.vector.memset(xq6[:, :], 1.0)
                nc.vector.tensor_copy(xq6[:LC, :], xq16[:LC, CT - 1, :])
                return xq16, xq6

            # ---- Phase B: stream Xk -> resident fp16 (DMA + DVE only);
            # xq block 0 is interleaved after jc 0 so the Xk stream (the
            # long pole for S0) starts first. ----
            xq_next = None
            for jc in range(NJ):
                js = slice(jc * JB, (jc + 1) * JB)
                for ct in range(CT - 1):
                    xf = xfp.tile([PT, JB], F32, tag="xstage",
                                  name=f"xkf{jc}_{ct}")
                    nc.sync.dma_start(
                        xf[:, :], Xk[ct * PT:(ct + 1) * PT, js])
                    nc.vector.tensor_copy(xk16[:, ct, js], xf[:, :])
                xt = xtp.tile([8, JB], F32, tag="xtail", name=f"xkt{jc}")
                nc.sync.dma_start(xt[:LC, :], Xk[(CT - 1) * PT:C, js])
                nc.vector.tensor_copy(xk16[:LC, CT - 1, js], xt[:LC, :])
                if jc == 1:
                    xq_next = load_xq(0)

            # prefetch xq block 1 BEFORE the XBAR gens occupy the sync
            # queue (they block on cast semaphores until ~2/3 through
            # phase B, which would delay block 1's A-projection)
            xq_pre = load_xq(1)

            # ---- QT: batched XBAR transposes (off the PE entirely) ----
            # Two halves per c-tile so the first half can fire as soon as
            # jc 0..3 have landed.  out[p, t, f] = in[f, 128*t + p].
            for half in (0, 1):
                hp = slice(half * (P // 2), (half + 1) * (P // 2))
                ht = slice(half * (IT // 2), (half + 1) * (IT // 2))
                for ct in range(CT):
                    pc = PT if ct < CT - 1 else 48
                    nc.sync.dma_start_transpose(
                        qt[:, ht, ct * PT:ct * PT + pc],
                        xk16[:pc, ct, hp],
                    )

            # ---- Phase D: fused A-projection + attention main loop ----
            with (
                tc.tile_pool(name="astp", bufs=2) as astp,
                tc.tile_pool(name="op", bufs=2) as op,
                tc.tile_pool(name="rp", bufs=1) as rp,
                tc.tile_pool(name="psA", bufs=2, space="PSUM") as psA,
                tc.tile_pool(name="psS", bufs=3, space="PSUM") as psS,
                tc.tile_pool(name="psO", bufs=3, space="PSUM") as psO,
            ):
                for jb in range(NJ):
                    js = slice(jb * JB, (jb + 1) * JB)
                    xq16, xq6 = xq_next
                    if jb == 0:
                        xq_next = xq_pre
                    elif jb < NJ - 1:
                        xq_next = load_xq(jb + 1)

                    # A-proj: A[:, jblock] = G @ Xq + w1 (ragged K=6 MM
                    # carries both the c2 tail and the bias row); the
                    # zero-padded lhsT makes all 128 psum rows valid
                    ast = astp.tile([PT, CT, JB], F16, tag="ast",
                                    name=f"ast{jb}")
                    for ot in range(CT):
                        ps = psA.tile([PT, JB], F32, tag="a",
                                      name=f"a{jb}_{ot}")
                        for ct2 in range(CT - 1):
                            nc.tensor.matmul(
                                ps[:, :],
                                g16[:, ct2, ot * PT:(ot + 1) * PT],
                                xq16[:, ct2, :],
                                start=(ct2 == 0),
                                stop=False,
                                skip_group_check=True,
                            )
                        nc.tensor.matmul(
                            ps[:, :],
                            g6[:LC + 1, ot * PT:(ot + 1) * PT],
                            xq6[:LC + 1, :],
                            start=False,
                            stop=True,
                            skip_group_check=True,
                        )
                        nc.any.tensor_copy(ast[:, ot, :], ps[:, :])

                    # S-phase: 32 chains of 7 K=128 MMs, exp into es
                    for t in range(IT):
                        ts = slice(t * PT, (t + 1) * PT)
                        ps_s = psS.tile([PT, JB], F32, tag="s",
                                        name=f"s{jb}_{t}")
                        for ct in range(CT):
                            nc.tensor.matmul(
                                ps_s[:, :],
                                xk16[:, ct, ts],
                                ast[:, ct, :],
                                start=(ct == 0),
                                stop=(ct == CT - 1),
                                skip_group_check=True,
                            )
                        nc.scalar.activation(
                            es[:, t, :], ps_s[:],
                            mybir.ActivationFunctionType.Exp, scale=SCALE,
                        )

                    # out-phase: chain kt=0 FIRST (its early MMs only need
                    # the early es tiles, hiding the exp tail), then the
                    # ragged chain (data rows 768..772 + sums row from the
                    # qt ones column) so the reciprocal + broadcast still
                    # overlap the remaining chains.  kt=0's normalization
                    # is deferred until bc exists (PSUM bank held).
                    bc = None
                    held = None

                    def normalize(kt, ps_o):
                        mo = LC if kt == CT - 1 else PT
                        osb = op.tile([PT, JB], F32, tag="osb",
                                      name=f"osb{jb}_{kt}")
                        nc.vector.tensor_mul(
                            out=osb[:mo, :], in0=ps_o[:mo, :],
                            in1=bc[:mo, :],
                        )
                        nc.sync.dma_start(
                            out[kt * PT:kt * PT + mo, js], osb[:mo, :])

                    for kt in (0, CT - 1) + tuple(range(1, CT - 1)):
                        mk = 33 if kt == CT - 1 else PT
                        ps_o = psO.tile([PT, JB], F32, tag="o",
                                        name=f"o{jb}_{kt}")
                        for t in range(IT):
                            nc.tensor.matmul(
                                ps_o[:mk, :],
                                qt[:, t, kt * PT:kt * PT + mk],
                                es[:, t, :],
                                start=(t == 0),
                                stop=(t == IT - 1),
                                skip_group_check=True,
                            )
                        if kt == CT - 1:
                            # fp16 1/den costs ~5e-4 relative error on the
                            # output, far under the 2e-2 gate; halves SBUF
                            recip = rp.tile([1, JB], F16, tag="recip")
                            with nc.allow_low_precision(
                                    reason="fp16 softmax denom recip"):
                                nc.vector.reciprocal(
                                    recip[:], ps_o[32:33, :])
                            bc = rp.tile([PT, JB], F16, tag="bc")
                            nc.gpsimd.partition_broadcast(bc[:], recip[:])
                        if bc is None:
                            held = (kt, ps_o)
                            continue
                        normalize(kt, ps_o)
                        if held is not None:
                            normalize(*held)
                            held = None

            xtp.release()
            xfp.release()
            xqp.release()
            qtp.release()

    nc.compile()
    return nc


_CACHE = {}


def _get_program(P=4096, n_cores=8):
    key = (P, n_cores)
    if key not in _CACHE:
        _CACHE[key] = build(P, n_cores)
    return _CACHE[key]


def _run(inputs, trace=False, **kw):
    nc = _get_program()
    Xq = np.asarray(inputs["Xq"], dtype=np.float32)
    Xk = np.asarray(inputs["Xk"], dtype=np.float32)
    Wk = np.ascontiguousarray(np.asarray(inputs["Wk"], dtype=np.float32))
    bkv = np.ascontiguousarray(np.asarray(inputs["bk"], dtype=np.float32))
    Wv = np.ascontiguousarray(np.asarray(inputs["Wv"], dtype=np.float32))
    bvv = np.ascontiguousarray(np.asarray(inputs["bv"], dtype=np.float32))
    B = Xq.shape[0]
    in_maps = [
        {
            "Xq": np.ascontiguousarray(Xq[b]),
            "Xk": np.ascontiguousarray(Xk[b]),
            "Wk": Wk,
            "bk": bkv,
            "Wv": Wv,
            "bv": bvv,
        }
        for b in range(B)
    ]
    res = run_bass_kernel_spmd(nc, in_maps, list(range(B)), trace=trace, **kw)
    outs = np.stack([res.results[b]["out"] for b in range(B)], axis=0)
    return outs.astype(np.float32), res


def kernel(**inputs):
    outs, _ = _run(inputs)
    return outs
